# revision 19
# baseline (speedup 1.0000x reference)
"""Dense attention kernel for Trainium2, 8 NeuronCores (SPMD).

Problem: q,k,v [8192, 1024] fp32; out = softmax(q @ k.T / sqrt(1024)) @ v.

Strategy (sequence-parallel over q, per the sharding hint):
  - Core c owns q rows [c*1024, (c+1)*1024); k and v are replicated.
  - Host pre-transposes: each core receives qT [D, M]=[1024, 1024] (its q
    shard transposed) and kT [D, N]=[1024, 8192] (k transposed), so the
    contraction dim D is the SBUF partition dim for both matmul operands
    and no on-chip transposes are needed anywhere.
  - Scores are computed TRANSPOSED: sT[n, m] = sum_d kT[d, n] * qT[d, m]
    (lhsT = kT chunk, rhs = qT chunk). The softmax numerator
    pT = exp(sT / 32) then already has the kv dim n on partitions, which is
    exactly the lhsT layout the second matmul needs: o[m, j] += pT.T @ v.
  - No running max: scores/32 ~ N(0,1), max over 8192 ~ 4.3, so exp() is
    bounded by ~e^5 — no overflow risk in fp32, and softmax is shift
    invariant so the result matches the reference.
  - The softmax denominator l[m] = sum_n pT[n, m] falls out of a 1-column
    matmul against a ones vector. l accumulates in PSUM across ALL kv
    blocks (two banks, even/odd m-tiles alternating) so no DVE staging is
    needed and the tail's reciprocal read of one bank never serializes
    against the PE writing the other.
  - In the last kv block, finalization is fused per m-tile and split
    across engines: ScalarE computes o_acc*rcp (Copy activation with a
    per-partition scale AP) while DVE does the fused (o_ps*rcp)+x via
    scalar_tensor_tensor, one 512-col half at a time with the store
    issued per half. Keeps the tail DVE chain under the PE m-tile
    cadence so nothing queues after the last matmul.

Ramp: the single model DMA queue drains in dma_start program order and
~8us of fixed runtime preamble precede the first descriptor, so the
critical set (qT first m-half + kT block 0) is issued as 4 interleaved
512 KB transfers — the first score matmuls unblock after 1 MB. Junk
matmuls on a zero tile (no DMA dependency) keep the PE busy from ~7.5us
so the HAM clock gate releases (K=8/8) before real data arrives;
without this the first ~15 real matmuls run at 1.2 GHz.

kv is streamed once per core in blocks of NB columns; o accumulates in
SBUF fp32 across blocks.

The executor mirrors concourse.bass2jax.run_bass_via_pjrt but caches the
jitted computation (run_bass_via_pjrt re-traces per call). `reps` unrolls
the whole attention pass inside the module for steady-state timing.

Measured (NTFF profile, core 0): 2048 512-col matmuls run at ~215 ns
(the warm bf16 streaming rate, 99% of peak); PE busy-union 455.8 us,
exec ~477 us (was 489-490 before the ramp/tail/l-bank work).
fp8/DoubleRow was evaluated and rejected: softmax output magnitude
scales with the same sqrt(sum w^2) factor as weight quantization noise,
so rel err ~= the raw fp8 quant error (~3-7%), over the 2e-2 gate.
"""

import numpy as np
import ml_dtypes

# ---- problem geometry (hardcoded per contract) ----
N = 8192
D = 1024
NCORES = 8
M = N // NCORES  # 1024 q rows per core

P = 128
DC = D // P  # 8 contraction chunks
NB = 512  # kv block columns (1024 measured slower on HW)
NBLK = N // NB  # 16
NCX = NB // P  # 4 partition-chunks of kv per block
MTS = M // P  # 8 m-tiles per core
MH = 512  # rhs stream width for the scores matmul
NMH = M // MH  # 2

# "bf16": cast q/k/v to bf16 on host, matmuls at full PE rate.
# "f32r": keep fp32 storage, matmuls in float32r (relaxed fp32) mode.
MM_DTYPE = "bf16"

# walrus's --enable-ldw-opt rejects bass-emitted InstLdweights outright
# ("InstLdweights is not compatible with LDW optimization"), so weight-load
# dedup must come from instruction ORDER instead: consecutive matmuls that
# share a stationary operand are placed back-to-back (measured ~107 ns
# cheaper per redundant load on HW).
ENABLE_LDW_OPT = False

# PSUM bank split (8 banks total): 2 score banks, 4 out-phase banks
# (j-half tiles; the 4th slot lets m-tile t+1's second j-half start before
# m-tile t's psum drain completes), 2 l banks (even/odd m-tiles).
SPS_BUFS = 2
OPS_BUFS = 4

# HAM pre-warm: junk matmuls at kernel start (no data dependency) so the
# PE clock gate is already at K=8/8 when the first real matmul's DMA
# lands (~17us: ~8us fixed runtime preamble + ~7us critical-set DMA +
# ~2us completion latency). First ~7 run cold (~630ns), the rest warm
# (~215ns) — together they bridge the PE to the first data-ready matmul
# (~13us). Ending early just idles the PE briefly; ending late delays
# real work, so bias low.
WARMUP_MMS = 16

SCALE = 1.0 / np.sqrt(np.float32(D))

_cache = {}


def _patch_ldw_opt():
    if _cache.get("ldw_patched") or not ENABLE_LDW_OPT:
        return
    from concourse import bass_utils

    orig = bass_utils.run_command

    def run_command_ldw(argv, **kwargs):
        argv = ["--enable-ldw-opt=true" if a == "--enable-ldw-opt=false" else a
                for a in argv]
        return orig(argv, **kwargs)

    bass_utils.run_command = run_command_ldw
    _cache["ldw_patched"] = True


def _build(mm_dtype, reps=1):
    import concourse.bass as bass
    import concourse.tile as tile
    import concourse.mybir as mybir
    from concourse import bacc

    f32 = mybir.dt.float32
    if mm_dtype == "bf16":
        mdt = mybir.dt.bfloat16
        mmcast = lambda ap: ap
    else:
        mdt = mybir.dt.float32
        mmcast = lambda ap: ap.bitcast(mybir.dt.float32r)

    # disable_frame_to_traceback keeps caller tracebacks out of the BIR so
    # the build (and the NEFF-cache key) is identical from any call site.
    nc = bacc.Bacc("TRN2", target_bir_lowering=False, debug=False,
                   num_devices=NCORES, disable_frame_to_traceback=True)
    qT_d = nc.declare_dram_parameter("qT", [D, M], mdt, isOutput=False)
    kT_d = nc.declare_dram_parameter("kT", [D, N], mdt, isOutput=False)
    v_d = nc.declare_dram_parameter("v", [N, D], mdt, isOutput=False)
    o_d = nc.declare_dram_parameter("o", [M, D], f32, isOutput=True)

    qT_r = qT_d.rearrange("(dc p) m -> p dc m", p=P)
    kT_r = kT_d.rearrange("(dc p) n -> p dc n", p=P)
    v_r = v_d.rearrange("(nb p) j -> p nb j", p=P)
    o_r = o_d.rearrange("(mt p) j -> p mt j", p=P)

    Exp = mybir.ActivationFunctionType.Exp
    Copy = mybir.ActivationFunctionType.Copy
    mult = mybir.AluOpType.mult
    add = mybir.AluOpType.add

    # fp32 tiles are 2x the size; shrink buffering to fit SBUF.
    wide = mm_dtype != "bf16"
    qabufs = 1 if (reps == 1 or wide) else 2
    kvbufs = 2 if wide else 3

    with tile.TileContext(nc) as tc:
        with (
            tc.tile_pool(name="const", bufs=1) as cpool,
            tc.tile_pool(name="qT", bufs=qabufs) as qpool,
            tc.tile_pool(name="acc", bufs=qabufs) as apool,
            tc.tile_pool(name="kT", bufs=kvbufs) as kpool,
            tc.tile_pool(name="v", bufs=kvbufs) as vpool,
            tc.tile_pool(name="pT", bufs=2) as ppool,
            # 4 bufs: o_out recycling is gated on the store-completion
            # semaphore (HBM write receipt ~2us after the read-out), so 3
            # slots at the ~1.9us m-tile cadence still stalls the last
            # m-tiles' finalize past the final matmul.
            tc.tile_pool(name="fin", bufs=4) as fpool,
            tc.tile_pool(name="sps", bufs=SPS_BUFS, space="PSUM") as spsum,
            tc.tile_pool(name="ops", bufs=OPS_BUFS, space="PSUM") as opsum,
            tc.tile_pool(name="lps", bufs=1, space="PSUM") as lpsum,
        ):
            ones = cpool.tile([P, 1], mdt)
            nc.vector.memset(ones[:], 1.0)

            if WARMUP_MMS:
                junk = cpool.tile([P, 512], mdt)
                nc.vector.memset(junk[:], 0.0)
                jps = opsum.tile([P, 512], f32, tag="ops")
                for i in range(WARMUP_MMS):
                    nc.tensor.matmul(
                        jps[:], mmcast(junk[:, 0:P]), mmcast(junk[:]),
                        start=(i == 0), stop=(i == WARMUP_MMS - 1),
                        skip_group_check=True,
                    )

            for _ in range(reps):
                # l accumulates in PSUM across the whole pass: even m-tiles
                # in one bank, odd in the other, so the tail's reciprocal
                # read of bank A overlaps PE writes to bank B.
                l_even = lpsum.tile([P, MTS // 2], f32, tag="le")
                l_odd = lpsum.tile([P, MTS // 2], f32, tag="lo")
                l_ps = [l_even, l_odd]

                qT_sb = qpool.tile([P, DC, M], mdt)
                kT_b0 = kpool.tile([P, DC, NB], mdt)
                # Critical-path DMA order (single model queue drains in
                # program order). Big coalesced transfers: 1 MB runs at
                # ~341 GB/s where 128 KB chunks only reach ~170 GB/s.
                # First score group needs qT's first m-half + kT block 0;
                # then v block 0 (first out-phase), then the second m-half.
                # qT first m-half and kT block 0 in interleaved dc-halves
                # (4 x 512 KB): the first score matmuls (dc<4) unblock after
                # just 1 MB instead of the full 2 MB critical set.
                hdc = DC // 2
                nc.sync.dma_start(qT_sb[:, 0:hdc, 0:MH], qT_r[:, 0:hdc, 0:MH])
                nc.sync.dma_start(kT_b0[:, 0:hdc, :], kT_r[:, 0:hdc, 0:NB])
                nc.sync.dma_start(qT_sb[:, hdc:, 0:MH], qT_r[:, hdc:, 0:MH])
                nc.sync.dma_start(kT_b0[:, hdc:, :], kT_r[:, hdc:, 0:NB])
                v_b0 = vpool.tile([P, NCX, D], mdt)
                nc.sync.dma_start(v_b0[:], v_r[:, 0:NCX, :])
                nc.sync.dma_start(qT_sb[:, :, MH:M], qT_r[:, :, MH:M])

                o_acc = apool.tile([P, MTS, D], f32)

                for b in range(NBLK):
                    last = b == NBLK - 1
                    if b == 0:
                        kT_blk, v_blk = kT_b0, v_b0
                    else:
                        kT_blk = kpool.tile([P, DC, NB], mdt)
                        nc.sync.dma_start(kT_blk[:],
                                          kT_r[:, :, b * NB:(b + 1) * NB])
                        v_blk = vpool.tile([P, NCX, D], mdt)
                        nc.sync.dma_start(v_blk[:],
                                          v_r[:, b * NCX:(b + 1) * NCX, :])

                    pT = ppool.tile([P, NCX, M], mdt)
                    for mh in range(NMH):
                        # mh-major: scores+exp for this m-half, then its
                        # out-phase m-tiles. In block 0 the out work for
                        # m-tiles 0-3 is then ready before qT's second
                        # m-half has even landed.
                        for ncx in range(NCX):
                            sT = spsum.tile([P, MH], f32, tag="sT1")
                            for dc in range(DC):
                                nc.tensor.matmul(
                                    sT[:],
                                    mmcast(kT_blk[:, dc, ncx * P:(ncx + 1) * P]),
                                    mmcast(qT_sb[:, dc, mh * MH:(mh + 1) * MH]),
                                    start=(dc == 0), stop=(dc == DC - 1),
                                )
                            nc.scalar.activation(
                                pT[:, ncx, mh * MH:(mh + 1) * MH], sT[:],
                                Exp, scale=float(SCALE),
                            )

                        for mt in range(mh * MTS // NMH, (mh + 1) * MTS // NMH):
                            lt = l_ps[mt % 2]
                            lc = mt // 2
                            o_ps0 = opsum.tile([P, 512], f32, tag="ops")
                            o_ps1 = opsum.tile([P, 512], f32, tag="ops")
                            rcp = o_out = None
                            if last:
                                # l-matmuls first: rcp + the ScalarE copy
                                # then overlap this m-tile's o-matmuls, so
                                # after the last matmul only the two fused
                                # adds remain. (Costs one extra hidden
                                # LDWEIGHTS per ncx — irrelevant.)
                                for ncx in range(NCX):
                                    nc.tensor.matmul(
                                        lt[:, lc:lc + 1],
                                        mmcast(pT[:, ncx, mt * P:(mt + 1) * P]),
                                        mmcast(ones[:]),
                                        start=False, stop=(ncx == NCX - 1),
                                        skip_group_check=True,
                                    )
                                rcp = fpool.tile([P, 1], f32, tag="rcp")
                                nc.vector.reciprocal(rcp[:], lt[:, lc:lc + 1])
                                o_out = fpool.tile([P, D], f32, tag="oout")
                                nc.scalar.activation(o_out[:], o_acc[:, mt, :],
                                                     Copy, scale=rcp[:])
                            if last:
                                # Phase-major in the last block: all of
                                # j-half 0, then j-half 1 — half 0's fused
                                # add overlaps half 1's matmuls, so only
                                # one DVE op trails the final matmul.
                                # (Extra hidden LDWEIGHTS per ncx.)
                                for ops, js in ((o_ps0, slice(0, 512)),
                                                (o_ps1, slice(512, 1024))):
                                    for ncx in range(NCX):
                                        nc.tensor.matmul(
                                            ops[:],
                                            mmcast(pT[:, ncx, mt * P:(mt + 1) * P]),
                                            mmcast(v_blk[:, ncx, js]),
                                            start=(ncx == 0),
                                            stop=(ncx == NCX - 1),
                                        )
                            else:
                                for ncx in range(NCX):
                                    pw = mmcast(pT[:, ncx, mt * P:(mt + 1) * P])
                                    nc.tensor.matmul(
                                        o_ps0[:], pw, mmcast(v_blk[:, ncx, 0:512]),
                                        start=(ncx == 0), stop=(ncx == NCX - 1),
                                    )
                                    nc.tensor.matmul(
                                        o_ps1[:], pw, mmcast(v_blk[:, ncx, 512:1024]),
                                        start=(ncx == 0), stop=(ncx == NCX - 1),
                                    )
                                    # start=True clears the whole bank, so
                                    # only the first m-tile touching each l
                                    # bank may set it; later m-tiles'
                                    # columns have has_written=0 and
                                    # overwrite correctly.
                                    nc.tensor.matmul(
                                        lt[:, lc:lc + 1], pw, mmcast(ones[:]),
                                        start=(b == 0 and ncx == 0 and mt < 2),
                                        stop=False,
                                        skip_group_check=True,
                                    )
                            halves = ((o_ps0, slice(0, 512)),
                                      (o_ps1, slice(512, 1024)))
                            if b == 0:
                                for ops, js in halves:
                                    nc.vector.tensor_copy(o_acc[:, mt, js],
                                                          ops[:])
                            elif not last:
                                for ops, js in halves:
                                    nc.vector.tensor_add(o_acc[:, mt, js],
                                                         o_acc[:, mt, js],
                                                         ops[:])
                            else:
                                # fused finalization, split across engines:
                                # ScalarE already ran o_out = o_acc * rcp;
                                # DVE per half: o_out += o_ps * rcp, store
                                # per half.
                                for ops, js in halves:
                                    nc.vector.scalar_tensor_tensor(
                                        o_out[:, js], ops[:], rcp[:],
                                        o_out[:, js], op0=mult, op1=add)
                                    nc.sync.dma_start(o_r[:, mt, js],
                                                      o_out[:, js])

    # Scrub residual caller tracebacks (Tile's exit path captures one even
    # with disable_frame_to_traceback) so the BIR — and therefore the NEFF
    # compile-cache key — is identical from any call site.
    import dataclasses
    for bb in nc.m.functions[0].blocks:
        for inst in bb.instructions:
            d = inst.debug
            if d is not None and d.ant_traceback is not None:
                inst.debug = dataclasses.replace(d, ant_traceback=None)

    nc.finalize()
    return nc


def _get_exec(reps=1):
    """Build (once) and cache a jitted SPMD executor whose module runs
    `reps` chained attention passes. Returns (fn, in_names, out_names,
    out_avals); fn(*global_inputs, *global_zero_outs) -> global outputs."""
    key = ("exec", MM_DTYPE, reps)
    if key in _cache:
        return _cache[key]

    import jax
    from jax.sharding import Mesh, PartitionSpec
    from jax.experimental.shard_map import shard_map
    import concourse.mybir as mybir
    from concourse import bass2jax

    nckey = ("nc", MM_DTYPE, reps)
    if nckey not in _cache:
        _cache[nckey] = _build(MM_DTYPE, reps)
    nc = _cache[nckey]

    _patch_ldw_opt()
    bass2jax.install_neuronx_cc_hook()

    partition_name = nc.partition_id_tensor.name if nc.partition_id_tensor else None
    in_names, out_names, out_avals = [], [], []
    for alloc in nc.m.functions[0].allocations:
        if not isinstance(alloc, mybir.MemoryLocationSet):
            continue
        name = alloc.memorylocations[0].name
        if alloc.kind == "ExternalInput":
            if name != partition_name:
                in_names.append(name)
        elif alloc.kind == "ExternalOutput":
            out_names.append(name)
            out_avals.append(jax.core.ShapedArray(
                tuple(alloc.tensor_shape), mybir.dt.np(alloc.dtype)))
    n_params = len(in_names)
    n_outs = len(out_names)
    bind_names = tuple(in_names + out_names + (
        [partition_name] if partition_name else []))

    def _body(*args):
        operands = list(args)
        if partition_name is not None:
            operands.append(bass2jax.partition_id_tensor())
        outs = bass2jax._bass_exec_p.bind(
            *operands,
            out_avals=tuple(out_avals),
            in_names=bind_names,
            out_names=tuple(out_names),
            lowering_input_output_aliases=(),
            sim_require_finite=True,
            sim_require_nnan=True,
            nc=nc,
        )
        return tuple(outs)

    devices = jax.devices()[:NCORES]
    mesh = Mesh(np.asarray(devices), ("core",))
    donate = tuple(range(n_params, n_params + n_outs))
    # qT is sharded along cores; kT and v are replicated (spec None), so the
    # host passes ONE copy instead of materializing 8.
    in_spec_map = {"qT": PartitionSpec("core"), "kT": PartitionSpec(),
                   "v": PartitionSpec()}
    fn = jax.jit(shard_map(
        _body, mesh=mesh,
        in_specs=tuple(in_spec_map[nm] for nm in in_names)
        + (PartitionSpec("core"),) * n_outs,
        out_specs=(PartitionSpec("core"),) * n_outs,
        check_rep=False,
    ), donate_argnums=donate, keep_unused=True)
    _cache[key] = (fn, in_names, out_names, out_avals)
    return _cache[key]


def _prep_inputs(q, k, v):
    """Per-core host preprocessing -> dict name -> global concat array."""
    npdt = ml_dtypes.bfloat16 if MM_DTYPE == "bf16" else np.float32
    # Cast BEFORE transposing: the transpose-copy then moves half the bytes.
    kb = np.asarray(k).astype(npdt)
    kT = np.ascontiguousarray(kb.T)
    vv = np.ascontiguousarray(np.asarray(v).astype(npdt))
    qb = np.asarray(q).astype(npdt)
    qT_g = np.ascontiguousarray(
        qb.reshape(NCORES, M, D).transpose(0, 2, 1)).reshape(NCORES * D, M)
    # kT and v are replicated by the executor (in_spec PartitionSpec()),
    # so a single copy suffices here.
    return {"qT": qT_g, "kT": kT, "v": vv}


def _device_zeros(out_avals):
    """Per-call donated output buffers, created on device (no host transfer)."""
    import jax
    import jax.numpy as jnp
    from jax.sharding import Mesh, NamedSharding, PartitionSpec

    if "zfn" not in _cache:
        mesh = Mesh(np.asarray(jax.devices()[:NCORES]), ("core",))
        shard = NamedSharding(mesh, PartitionSpec("core"))
        shapes = [((NCORES * a.shape[0], *a.shape[1:]), a.dtype)
                  for a in out_avals]
        _cache["zfn"] = jax.jit(
            lambda: tuple(jnp.zeros(s, d) for s, d in shapes),
            out_shardings=(shard,) * len(shapes))
    return _cache["zfn"]()


def kernel(q, k, v):
    fn, in_names, out_names, out_avals = _get_exec(reps=1)
    global_ins = _prep_inputs(q, k, v)
    outs = fn(*[global_ins[nm] for nm in in_names], *_device_zeros(out_avals))
    o = np.asarray(outs[out_names.index("o")])
    return o.reshape(NCORES * M, D)
